# revision 10
# baseline (speedup 1.0000x reference)
"""Trainium2 Bass kernel for nn_DEERForward (quasi-DEER fixed-point solver).

Self-contained: accepts FULL inputs, shards tokens over 8 NeuronCores,
runs a fused single-weight-pass fp16 kernel, gathers the full output.

Algorithm restructure (validated against the jax reference to ~1e-5 in f64):
  - warm, iter0 and iter1 block applications for layer l are fused into one
    pass so W1_l / W2_l stream from HBM exactly once.
  - The FD probe (op - om) is factored: op-om = 2*eps*v + (gelu(zp)-gelu(zm))@W2
    with zp/zm built in PSUM as  z +- (dxn_half @ W1)  where dxn_half =
    (LN(x+ev)-LN(x-ev))/2 is computed analytically from joint LN stats.
    This keeps full relative precision on the small FD difference even with
    11-bit matmul operands.
  - layer-0 residuals are identically zero (outs_0 == warm_0), so the
    iteration corrections start at layer 1.
Layout: activations stay transposed [C on partitions, tokens free] end to
end -> both matmuls use natural weight layouts as the stationary operand and
no transposes are needed anywhere.

hblk per-F-tile block layout (640 cols, f16): [h_w | h_i0 | h_i1 | d0 | d1];
m2 consumes [0:512] ([h_w|h_i0|h_i1|d0], N=512) and [512:640] (d1, N=128).
stage per-C-tile block layout (640 cols, f32):
  [mlp_w | mlp_i0 | mlp_i1 | dW2_0 | dW2_1]
"""
import numpy as np
from contextlib import ExitStack

import concourse.bass as bass
from concourse import bacc, tile, mybir
from concourse.alu_op_type import AluOpType as ALU
from concourse.bass_utils import run_bass_kernel_spmd

dt = mybir.dt
AF = mybir.ActivationFunctionType

L, C, F, TOK = 12, 768, 3072, 128
NC6, NF = C // 128, F // 128       # 6, 24
NCORES = 8
LN_EPS = float(np.float32(1e-5))
EPSF = float(np.float32(1e-3))               # FD eps as fp32 value
EPS2 = float(np.float64(EPSF) * np.float64(EPSF))
K2 = float(1.0 / (2.0 * np.float64(EPSF) ** 2))   # diag_J = dW2*ev*K2 + 1
INV_SCALE = 1024.0 / 768.0                   # stats lhsT holds 2^-10 exactly

F16, F32 = dt.float16, dt.float32


def _emit_stats(nc, ps, smol, statrhs, ones_k, eps_tile, have_ev, uid):
    """Stats matmul + finalize. statrhs: [128, 6, nb, 128] f16 with blocks
    [s, s^2, (ev, s*ev)]. Returns dict of [1,128] f32 APs with LN stats."""
    nb = 4 if have_ev else 2
    pst = ps.tile([1, 512], F32, tag="psA", name=f"pst{uid}")
    for c in range(NC6):
        nc.tensor.matmul(pst[:, 0:nb * 128], ones_k[:, 0:1],
                         statrhs[:, c, 0:nb, :], start=(c == 0),
                         stop=(c == NC6 - 1))
    st = smol.tile([1, 4, 128], F32, tag="st", name=f"st{uid}")
    nc.vector.tensor_scalar(
        st[:, 0:nb, :], pst[:, 0:nb * 128].rearrange("p (b t) -> p b t", b=nb),
        INV_SCALE, None, ALU.mult)
    mu = st[:, 0, :]
    e2 = st[:, 1, :]
    out = {"mu": mu, "st_tile": st}
    var3 = smol.tile([1, 3, 128], F32, tag="var3", name=f"var3{uid}")
    t0 = smol.tile([1, 128], F32, tag="t0", name=f"t0{uid}")
    nc.vector.tensor_mul(t0[:], mu, mu)
    nc.vector.tensor_sub(var3[:, 0, :], e2, t0[:])
    nvar = 1
    if have_ev:
        me, mxv = st[:, 2, :], st[:, 3, :]
        mup = smol.tile([1, 128], F32, tag="mup", name=f"mup{uid}")
        mum = smol.tile([1, 128], F32, tag="mum", name=f"mum{uid}")
        nc.vector.tensor_add(mup[:], mu, me)
        nc.vector.tensor_sub(mum[:], mu, me)
        tpm = smol.tile([1, 2, 128], F32, tag="tpm", name=f"tpm{uid}")
        # E[(x+-ev)^2] = e2 +- 2*mxv + eps^2
        nc.vector.tensor_scalar(tpm[:, 0, :], mxv, 2.0, EPS2, ALU.mult, ALU.add)
        nc.vector.tensor_add(tpm[:, 0, :], tpm[:, 0, :], e2)
        nc.vector.tensor_scalar(tpm[:, 1, :], mxv, -2.0, EPS2, ALU.mult, ALU.add)
        nc.vector.tensor_add(tpm[:, 1, :], tpm[:, 1, :], e2)
        t1 = smol.tile([1, 128], F32, tag="t1", name=f"t1{uid}")
        nc.vector.tensor_mul(t1[:], mup[:], mup[:])
        nc.vector.tensor_sub(var3[:, 1, :], tpm[:, 0, :], t1[:])
        nc.vector.tensor_mul(t1[:], mum[:], mum[:])
        nc.vector.tensor_sub(var3[:, 2, :], tpm[:, 1, :], t1[:])
        out["mup"], out["mum"] = mup, mum
        nvar = 3
    rs3 = smol.tile([1, 3, 128], F32, tag="rs3", name=f"rs3{uid}")
    nc.scalar.activation(rs3[:, 0:nvar, :], var3[:, 0:nvar, :], AF.Sqrt,
                         bias=eps_tile[:, 0:1], scale=1.0)
    nc.vector.reciprocal(rs3[:, 0:nvar, :], rs3[:, 0:nvar, :])
    out["rs"] = rs3[:, 0, :]
    if have_ev:
        out["rsp"], out["rsm"] = rs3[:, 1, :], rs3[:, 2, :]
    return out


def _build(debug=False):
    nc = bacc.Bacc(None, target_bir_lowering=False, debug=False)
    xt = nc.dram_tensor("xt", [C, TOK], F32, kind="ExternalInput").ap()
    evd = nc.dram_tensor("ev", [2, L, C, TOK], F32, kind="ExternalInput").ap()
    w1d = nc.dram_tensor("w1", [L, C, F], F16, kind="ExternalInput").ap()
    w2d = nc.dram_tensor("w2", [L, F, C], F16, kind="ExternalInput").ap()
    b1d = nc.dram_tensor("b1", [L, F], F32, kind="ExternalInput").ap()
    b2d = nc.dram_tensor("b2", [L, C], F32, kind="ExternalInput").ap()
    ytd = nc.dram_tensor("yt", [C, TOK], F32, kind="ExternalOutput").ap()
    dbg = {}
    if debug:
        def _dbgt(name, shape, dtp):
            dbg[name] = nc.dram_tensor(f"dbg_{name}", shape, dtp,
                                       kind="ExternalOutput").ap()
        _dbgt("xn0", [128, NC6, 128], F16)
        _dbgt("xnblk1", [128, NC6, 3, 128], F16)
        _dbgt("dblk1", [128, NC6, 4, 128], F16)
        _dbgt("stage1", [128, NC6, 640], F32)
        _dbgt("warm0", [128, NC6, 128], F32)
        _dbgt("warm1", [128, NC6, 128], F32)
        _dbgt("res0_1", [128, NC6, 128], F32)
        _dbgt("a0_1", [128, NC6, 128], F32)
        _dbgt("snn_1", [128, NC6, 128], F32)
        _dbgt("res1_1", [128, NC6, 128], F32)
        _dbgt("d1_1", [128, NC6, 128], F32)
        _dbgt("st1", [1, 4, 128], F32)
        _dbgt("scal1", [1, 5, 128], F16)
        _dbgt("hblk1", [128, NF, 640], F16)

    with tile.TileContext(nc) as tc, ExitStack() as ctx:
        consts = ctx.enter_context(tc.tile_pool(name="consts", bufs=1))
        wpool = ctx.enter_context(tc.tile_pool(name="wpool", bufs=2))
        spool = ctx.enter_context(tc.tile_pool(name="spool", bufs=2))
        hpool = ctx.enter_context(tc.tile_pool(name="hpool", bufs=1))
        smol = ctx.enter_context(tc.tile_pool(name="smol", bufs=2))
        ps = ctx.enter_context(tc.tile_pool(name="ps", bufs=2, space="PSUM"))

        ones_k = consts.tile([128, 1], F16, name="ones_k")
        nc.vector.memset(ones_k[:], 2.0 ** -10)
        ones_m = consts.tile([1, 128], F16, name="ones_m")
        nc.vector.memset(ones_m[:], 1.0)
        eps_tile = consts.tile([1, 1], F32, name="eps_tile")
        nc.vector.memset(eps_tile[:], LN_EPS)

        x0sb = consts.tile([128, NC6, 128], F32, name="x0sb")
        nc.sync.dma_start(x0sb[:], bass.AP(
            tensor=xt.tensor, offset=xt.offset,
            ap=[[TOK, 128], [128 * TOK, NC6], [1, TOK]]))

        # ---------- bootstrap: LN(x0) -> xn0 (f16) ----------
        xn0 = consts.tile([128, NC6, 128], F16, name="xn0")
        statrhs0 = smol.tile([128, NC6, 4, 128], F16, tag="statrhs", name="statrhs0", bufs=1)
        nc.scalar.copy(statrhs0[:, :, 0, :], x0sb[:])
        nc.vector.tensor_mul(statrhs0[:, :, 1, :], x0sb[:], x0sb[:])
        s0 = _emit_stats(nc, ps, smol, statrhs0, ones_k, eps_tile, False, "x0")
        scal0 = smol.tile([1, 5, 128], F16, tag="scal", name="scal0")
        nc.vector.tensor_copy(scal0[:, 0, :], s0["rs"])
        negmu0 = smol.tile([1, 128], F32, tag="negmu", name="negmu0")
        nc.vector.tensor_scalar(negmu0[:], s0["mu"], -1.0, None, ALU.mult)
        nc.vector.tensor_mul(scal0[:, 1, :], negmu0[:], s0["rs"])
        bc0 = ps.tile([128, 256], F32, tag="psB", name="bc0")
        nc.tensor.matmul(bc0[:], ones_m[:],
                         scal0[:, 0:2, :].rearrange("p b t -> p (b t)"),
                         start=True, stop=True)
        xtmp0 = spool.tile([128, NC6, 128], F32, tag="res", name="xn0tmp")
        nc.vector.tensor_tensor(
            xtmp0[:], x0sb[:],
            bc0[:, 0:128].unsqueeze(1).broadcast_to([128, NC6, 128]), ALU.mult)
        nc.vector.tensor_tensor(
            xn0[:], xtmp0[:],
            bc0[:, 128:256].unsqueeze(1).broadcast_to([128, NC6, 128]), ALU.add)

        if debug:
            nc.sync.dma_start(dbg["xn0"], xn0[:])
        s_old = None      # warm_{l-1} (= states_l^{(0)}), f32 tile
        s_new = None      # states_l^{(1)}, f32 tile
        d0_prev = None
        d1_prev = None

        for l in range(L):
            # ---------- weights + per-layer constants ----------
            w1q = []
            for q in range(4):
                t = wpool.tile([128, NC6, 6, 128], F16, tag="w1q", name=f"w1_{l}_{q}")
                for c in range(NC6):
                    nc.sync.dma_start(
                        t[:, c, :, :],
                        w1d[l, c * 128:(c + 1) * 128, q * 768:(q + 1) * 768]
                        .rearrange("p (f t) -> p f t", f=6))
                w1q.append(t)
            w2full = wpool.tile([128, NF, NC6, 128], F16, tag="w2f", bufs=1,
                                name=f"w2_{l}")
            for f in range(NF):
                nc.sync.dma_start(
                    w2full[:, f, :, :],
                    w2d[l, f * 128:(f + 1) * 128, :]
                    .rearrange("p (c t) -> p c t", c=NC6))

            def w1t(c, f):
                return w1q[f // 6][:, c, f % 6, :]

            def w2t(f, c):
                return w2full[:, f, c, :]

            b1sb = smol.tile([128, NF], F32, tag="b1sb", name=f"b1_{l}")
            nc.sync.dma_start(b1sb[:], bass.AP(
                tensor=b1d.tensor, offset=b1d.offset + l * F,
                ap=[[1, 128], [128, NF]]))
            b2sb = smol.tile([128, NC6], F32, tag="b2sb", name=f"b2_{l}")
            nc.sync.dma_start(b2sb[:], bass.AP(
                tensor=b2d.tensor, offset=b2d.offset + l * C,
                ap=[[1, 128], [128, NC6]]))
            b2bc = b2sb[:].unsqueeze(2).broadcast_to([128, NC6, 128])

            ev0 = ev1 = None
            xnblk = dblk = None
            if l > 0:
                ev0 = spool.tile([128, NC6, 128], F32, tag="ev0", name=f"ev0_{l}")
                ev1 = spool.tile([128, NC6, 128], F32, tag="ev1", name=f"ev1_{l}")
                for it, evt in ((0, ev0), (1, ev1)):
                    nc.sync.dma_start(evt[:], bass.AP(
                        tensor=evd.tensor,
                        offset=evd.offset + (it * L + l) * C * TOK,
                        ap=[[TOK, 128], [128 * TOK, NC6], [1, TOK]]))

                # xnblk: per C-tile rhs blocks [xn0 | xn_i0 | xn_i1]
                xnblk = smol.tile([128, NC6, 3, 128], F16, tag="xnblk",
                                  name=f"xnblk_{l}", bufs=1)
                # dblk: per C-tile FD blocks [dx0 | -dx0 | dx1 | -dx1]
                dblk = smol.tile([128, NC6, 4, 128], F16, tag="dblk",
                                 name=f"dblk_{l}", bufs=1)
                nc.vector.tensor_copy(xnblk[:, :, 0, :], xn0[:])

                for it, s_in, evt in ((0, s_old, ev0), (1, s_new, ev1)):
                    statrhs = smol.tile([128, NC6, 4, 128], F16, tag="statrhs",
                                        name=f"statrhs_{l}_{it}", bufs=1)
                    nc.scalar.copy(statrhs[:, :, 0, :], s_in[:])
                    nc.vector.tensor_mul(statrhs[:, :, 1, :], s_in[:], s_in[:])
                    nc.scalar.copy(statrhs[:, :, 2, :], evt[:])
                    nc.vector.tensor_mul(statrhs[:, :, 3, :], s_in[:], evt[:])
                    sst = _emit_stats(nc, ps, smol, statrhs, ones_k, eps_tile,
                                      True, f"{l}_{it}")
                    # scal blocks: [A, B, Dr, Sr, Bd]
                    scal = smol.tile([1, 5, 128], F16, tag="scal",
                                     name=f"scal_{l}_{it}")
                    nc.vector.tensor_copy(scal[:, 0, :], sst["rs"])
                    nmu = smol.tile([1, 128], F32, tag="negmu", name=f"nmu_{l}_{it}")
                    nc.vector.tensor_scalar(nmu[:], sst["mu"], -1.0, None, ALU.mult)
                    nc.vector.tensor_mul(scal[:, 1, :], nmu[:], sst["rs"])
                    td = smol.tile([1, 128], F32, tag="td", name=f"td_{l}_{it}")
                    nc.vector.tensor_sub(td[:], sst["rsp"], sst["rsm"])
                    nc.vector.tensor_scalar(scal[:, 2, :], td[:], 0.5, None, ALU.mult)
                    nc.vector.tensor_add(td[:], sst["rsp"], sst["rsm"])
                    nc.vector.tensor_scalar(scal[:, 3, :], td[:], 0.5, None, ALU.mult)
                    u1 = smol.tile([1, 128], F32, tag="u1", name=f"u1_{l}_{it}")
                    nc.vector.tensor_mul(u1[:], sst["mup"], sst["rsp"])
                    u2 = smol.tile([1, 128], F32, tag="u2", name=f"u2_{l}_{it}")
                    nc.vector.tensor_mul(u2[:], sst["mum"], sst["rsm"])
                    nc.vector.tensor_sub(u2[:], u2[:], u1[:])
                    nc.vector.tensor_scalar(scal[:, 4, :], u2[:], 0.5, None, ALU.mult)
                    # broadcast the 5 scalars to [128, token] via K=1 matmuls
                    if debug and l == 1 and it == 0:
                        nc.sync.dma_start(dbg["st1"], sst["st_tile"][:])
                        nc.sync.dma_start(dbg["scal1"], scal[:])
                    bct = ps.tile([128, 512], F32, tag="psB" if it == 0 else "psC1",
                                  name=f"bc_{l}_{it}")
                    bc2t = ps.tile([128, 128], F32, tag="psC2", name=f"bc2_{l}_{it}")
                    nc.tensor.matmul(bct[:], ones_m[:],
                                     scal[:, 0:4, :].rearrange("p b t -> p (b t)"),
                                     start=True, stop=True)
                    nc.tensor.matmul(bc2t[:], ones_m[:], scal[:, 4, :],
                                     start=True, stop=True)

                    def bcb(i, bct=bct, bc2t=bc2t):
                        if i < 4:
                            return bct[:, i * 128:(i + 1) * 128]\
                                .unsqueeze(1).broadcast_to([128, NC6, 128])
                        return bc2t[:].unsqueeze(1).broadcast_to([128, NC6, 128])

                    # xn = s*A + B
                    xtmp = spool.tile([128, NC6, 128], F32, tag="res",
                                      name=f"xtmp_{l}_{it}")
                    nc.vector.tensor_tensor(xtmp[:], s_in[:], bcb(0), ALU.mult)
                    nc.vector.tensor_tensor(xnblk[:, :, it + 1, :], xtmp[:],
                                            bcb(1), ALU.add)
                    # dxn_half = s*Dr + ev*Sr + Bd -> dblk[2it], negated -> [2it+1]
                    dtmp = spool.tile([128, NC6, 128], F32, tag="aj",
                                      name=f"dtmp_{l}_{it}")
                    nc.vector.tensor_tensor(dtmp[:], s_in[:], bcb(2), ALU.mult)
                    dtmp2 = spool.tile([128, NC6, 128], F32, tag="res2",
                                       name=f"dtmp2_{l}_{it}")
                    nc.vector.tensor_tensor(dtmp2[:], evt[:], bcb(3), ALU.mult)
                    nc.vector.tensor_add(dtmp[:], dtmp[:], dtmp2[:])
                    nc.vector.tensor_tensor(dblk[:, :, 2 * it, :], dtmp[:],
                                            bcb(4), ALU.add)
                    nc.vector.tensor_scalar(dblk[:, :, 2 * it + 1, :],
                                            dblk[:, :, 2 * it, :],
                                            -1.0, None, ALU.mult)

            if debug and l == 1:
                nc.sync.dma_start(dbg["xnblk1"], xnblk[:])
                nc.sync.dma_start(dbg["dblk1"], dblk[:])
            # ---------- m1 + gelu ----------
            hblk = hpool.tile([128, NF, 640], F16, tag="hblk", name=f"hblk_{l}")
            for f in range(NF):
                if l == 0:
                    psA = ps.tile([128, 384], F32, tag="psA", name=f"psA_{l}_{f}")
                    for c in range(NC6):
                        nc.tensor.matmul(psA[:, 0:128], w1t(c, f), xn0[:, c, :],
                                         start=(c == 0), stop=(c == NC6 - 1))
                    nc.scalar.activation(hblk[:, f, 0:128], psA[:, 0:128],
                                         AF.Gelu_apprx_tanh,
                                         bias=b1sb[:, f:f + 1], scale=1.0)
                    continue
                psA = ps.tile([128, 384], F32, tag="psA", name=f"psA_{l}_{f}")
                psB = ps.tile([128, 512], F32, tag="psB", name=f"psB_{l}_{f}")
                for c in range(NC6):
                    st_, sp_ = (c == 0), (c == NC6 - 1)
                    xnb_c = xnblk[:, c, :, :].rearrange("p b t -> p (b t)")
                    dup_c = bass.AP(tensor=xnb_c.tensor, offset=xnb_c.offset + 128,
                                    ap=[xnb_c.ap[0], [128, 2], [0, 2], [1, 128]])
                    dbl_c = dblk[:, c, :, :].rearrange("p b t -> p (b t)")
                    nc.tensor.matmul(psA[:], w1t(c, f), xnb_c, start=st_, stop=sp_)
                    nc.tensor.matmul(psB[:], w1t(c, f), dup_c,
                                     start=st_, stop=False, skip_group_check=True)
                    nc.tensor.matmul(psB[:], w1t(c, f), dbl_c,
                                     start=False, stop=sp_, skip_group_check=True)
                nc.scalar.activation(hblk[:, f, 0:384], psA[:], AF.Gelu_apprx_tanh,
                                     bias=b1sb[:, f:f + 1], scale=1.0)
                # [hp0|hm0|hp1|hm1] -> scratch, then d-subs into hblk
                hscr = smol.tile([128, 512], F32, tag="hscr", bufs=3,
                                 name=f"hscr_{l}_{f}")
                nc.scalar.activation(hscr[:], psB[:], AF.Gelu_apprx_tanh,
                                     bias=b1sb[:, f:f + 1], scale=1.0)
                nc.vector.tensor_tensor(hblk[:, f, 384:512], hscr[:, 0:128],
                                        hscr[:, 128:256], ALU.subtract)
                nc.vector.tensor_tensor(hblk[:, f, 512:640], hscr[:, 256:384],
                                        hscr[:, 384:512], ALU.subtract)

            # ---------- m2 ----------
            stage = hpool.tile([128, NC6, 640], F32, tag="stage", name=f"stage_{l}")
            for c in range(NC6):
                if l == 0:
                    psC1 = ps.tile([128, 512], F32, tag="psC1", name=f"psC1_{l}_{c}")
                    for f in range(NF):
                        nc.tensor.matmul(psC1[:, 0:128], w2t(f, c), hblk[:, f, 0:128],
                                         start=(f == 0), stop=(f == NF - 1))
                    nc.scalar.copy(stage[:, c, 0:128], psC1[:, 0:128])
                    continue
                psC1 = ps.tile([128, 512], F32, tag="psC1", name=f"psC1_{l}_{c}")
                psC2 = ps.tile([128, 128], F32, tag="psC2", name=f"psC2_{l}_{c}")
                for f in range(NF):
                    st_, sp_ = (f == 0), (f == NF - 1)
                    nc.tensor.matmul(psC1[:], w2t(f, c), hblk[:, f, 0:512],
                                     start=st_, stop=sp_)
                    nc.tensor.matmul(psC2[:], w2t(f, c), hblk[:, f, 512:640],
                                     start=st_, stop=sp_)
                nc.scalar.copy(stage[:, c, 0:512], psC1[:])
                nc.scalar.copy(stage[:, c, 512:640], psC2[:])

            if debug and l == 1:
                nc.sync.dma_start(dbg["stage1"], stage[:])
                nc.sync.dma_start(dbg["hblk1"], hblk[:])
            # ---------- epilogue (batched over C tiles) ----------
            warm = spool.tile([128, NC6, 128], F32, tag="warm", name=f"warm_{l}")
            nc.vector.tensor_tensor(warm[:], stage[:, :, 0:128], x0sb[:], ALU.add)
            nc.vector.tensor_tensor(warm[:], warm[:], b2bc, ALU.add)
            if debug and l <= 1:
                nc.sync.dma_start(dbg[f"warm{l}"], warm[:])
            if l == 0:
                s_old, s_new = warm, warm
                continue

            res0 = spool.tile([128, NC6, 128], F32, tag="res", name=f"res0_{l}")
            nc.vector.tensor_tensor(res0[:], s_old[:], b2bc, ALU.add)
            nc.vector.tensor_sub(res0[:], res0[:], warm[:])
            nc.vector.tensor_tensor(res0[:], res0[:], stage[:, :, 128:256], ALU.add)
            a0 = spool.tile([128, NC6, 128], F32, tag="aj", name=f"a0_{l}")
            nc.vector.tensor_tensor(a0[:], stage[:, :, 384:512], ev0[:], ALU.mult)
            nc.vector.tensor_scalar(a0[:], a0[:], K2, 1.0, ALU.mult, ALU.add)
            d0 = spool.tile([128, NC6, 128], F32, tag="d0", name=f"d0_{l}")
            if l == 1:
                nc.vector.tensor_copy(d0[:], res0[:])
            else:
                nc.vector.tensor_mul(d0[:], a0[:], d0_prev[:])
                nc.vector.tensor_add(d0[:], d0[:], res0[:])
            snn = spool.tile([128, NC6, 128], F32, tag="snew", name=f"snn_{l}")
            nc.vector.tensor_add(snn[:], warm[:], d0[:])
            if debug and l == 1:
                nc.sync.dma_start(dbg["res0_1"], res0[:])
                nc.sync.dma_start(dbg["a0_1"], a0[:])
                nc.sync.dma_start(dbg["snn_1"], snn[:])

            res1 = spool.tile([128, NC6, 128], F32, tag="res2", name=f"res1_{l}")
            nc.vector.tensor_tensor(res1[:], s_new[:], b2bc, ALU.add)
            nc.vector.tensor_sub(res1[:], res1[:], snn[:])
            nc.vector.tensor_tensor(res1[:], res1[:], stage[:, :, 256:384], ALU.add)
            a1 = spool.tile([128, NC6, 128], F32, tag="aj", name=f"a1_{l}")
            nc.vector.tensor_tensor(a1[:], stage[:, :, 512:640], ev1[:], ALU.mult)
            nc.vector.tensor_scalar(a1[:], a1[:], K2, 1.0, ALU.mult, ALU.add)
            d1 = spool.tile([128, NC6, 128], F32, tag="d1", name=f"d1_{l}")
            if l == 1:
                nc.vector.tensor_copy(d1[:], res1[:])
            else:
                nc.vector.tensor_mul(d1[:], a1[:], d1_prev[:])
                nc.vector.tensor_add(d1[:], d1[:], res1[:])

            if debug and l == 1:
                nc.sync.dma_start(dbg["res1_1"], res1[:])
                nc.sync.dma_start(dbg["d1_1"], d1[:])
            if l == L - 1:
                sfin = spool.tile([128, NC6, 128], F32, tag="snew", name="sfin")
                nc.vector.tensor_add(sfin[:], snn[:], d1[:])
                nc.sync.dma_start(bass.AP(
                    tensor=ytd.tensor, offset=ytd.offset,
                    ap=[[TOK, 128], [128 * TOK, NC6], [1, TOK]]), sfin[:])

            s_old, s_new = warm, snn
            d0_prev, d1_prev = d0, d1

    nc.compile()
    return nc


_NC_CACHE = []


def kernel(x0, w1, b1, w2, b2, g, beta):
    x0 = np.asarray(x0)
    B, T, Cin = x0.shape
    assert (Cin, w1.shape[0]) == (C, L)
    # ---- host preprocessing (f64): fold g into w1, beta@w1 + b1 into b1 ----
    w1f = (np.asarray(g, np.float64)[:, :, None] * np.asarray(w1, np.float64))
    b1f = np.einsum("lc,lcf->lf", np.asarray(beta, np.float64),
                    np.asarray(w1, np.float64)) + np.asarray(b1, np.float64)
    w1h = w1f.astype(np.float16)
    w2h = np.asarray(w2, np.float64).astype(np.float16)
    b1f = b1f.astype(np.float32)
    b2f = np.asarray(b2, np.float32)

    # FD probes (iter 0/1), eps-scaled, fp32 — must match the reference's RNG
    import jax
    import jax.numpy as jnp
    with jax.default_device(jax.devices("cpu")[0]):
        key = jax.random.key(42)
        evs = []
        for it in range(2):
            v = jnp.sign(jax.random.normal(jax.random.fold_in(key, it),
                                           (L, B, T, C), jnp.float32))
            evs.append(np.asarray(v, np.float32) * np.float32(1e-3))
    ev = np.stack(evs)                                   # (2, L, B, T, C)

    NT = B * T
    TPC = NT // NCORES                                   # tokens per core
    assert TPC == TOK
    xflat = np.asarray(x0, np.float32).reshape(NT, C)
    evflat = ev.reshape(2, L, NT, C)

    if not _NC_CACHE:
        _NC_CACHE.append(_build())
    nc = _NC_CACHE[0]

    in_maps = []
    for r in range(NCORES):
        sl = slice(r * TPC, (r + 1) * TPC)
        in_maps.append({
            "xt": np.ascontiguousarray(xflat[sl].T),
            "ev": np.ascontiguousarray(evflat[:, :, sl].transpose(0, 1, 3, 2)),
            "w1": w1h, "w2": w2h, "b1": b1f, "b2": b2f,
        })
    res = run_bass_kernel_spmd(nc, in_maps, core_ids=list(range(NCORES)))
    out = np.empty((NT, C), np.float32)
    for r in range(NCORES):
        out[r * TPC:(r + 1) * TPC] = res.results[r]["yt"].T
    return out.reshape(B, T, C)
